# revision 1
# baseline (speedup 1.0000x reference)
"""Trainium2 Bass kernel for the CNF reversible backward solve.

Math restructuring (exact, validated in fp64 against the jax reference):

The per-step recursion is tracked purely in H-space (H=256) via
Z = W1 z, Y = W1 y:
    a_even = tanh(Y + beta_even)
    Z     += Mz @ a_even                       (Mz = -h W1 W2)
    a_odd  = tanh(Z + beta_odd)
    Y'     = inv_l Y + (1-inv_l) Z + inv_l Mz @ a_odd

On device both states live in PSUM banks and are updated by matmuls only
(biases ride in via tiny rank-2 matmuls; the Y carry uses the normalization
Yhat = l*(Y + beta_even), making all step weights constant, with the carry
term p = inv_l*Yhat + (l-1)*Z entering through compensated-bf16 identity
matmuls p = p_hi + p_lo). The scalar engine therefore does exactly one tanh
per MLP eval, and everything except tanh->matmul->tanh is off the critical
chain.

Each core runs TWO independent 16-sample chains interleaved, so each
engine's dependency stalls on one chain are filled with the other chain's
work.

The device streams all activations a_e to DRAM; the D-space outputs are
exact fp64 host-side postprocessing:
    y_final = c_y y1 + sum_e gamma_e (W2 @ a_e) + c_b b2
    I_final = h (N sum(c) - sum_s c . a_even_s^2),   c = diag(W1 W2)

Sharding: data-parallel, B=256 -> 32 samples on each of 8 cores (2 chains
of 16); parameters replicated; gather + assembly on host.
"""

import numpy as np
import ml_dtypes
from contextlib import ExitStack

import concourse.bass as bass
import concourse.tile as tile
from concourse import bacc, mybir
from concourse.bass_utils import run_bass_kernel_spmd

# Problem constants (hardcoded per contract)
NCORES = 8
B, D, H = 256, 64, 256
NSTEP = 64
HSTEP = 1.0 / NSTEP
LCOUP = 0.999
INVL = 1.0 / LCOUP
BS = B // NCORES  # 32 samples per core
NSH = 1  # chains per core (2-chain interleave measured slower: scheduler serializes)
BSH = BS // NSH  # samples per chain
NBLK = H // 128  # 2 h-blocks
FREE = NBLK * BSH  # 32: free size of H-space tiles, layout (blk, sample)
NEVAL = 2 * NSTEP  # 128
DMA_CHUNKS = 4
CSTEPS = NSTEP // DMA_CHUNKS  # steps per out-DMA chunk
CCOLS = CSTEPS * FREE
ACOLS = NSTEP * FREE  # columns in each activation stream (per chain)

F32 = mybir.dt.float32
BF16 = mybir.dt.bfloat16
BF16NP = ml_dtypes.bfloat16

SHARED_INPUTS = [
    "w1t", "w1tl", "mzt", "mzl", "ib16", "dbz", "dby", "dbz0", "dby0", "ind", "indb",
]


def _coefficients():
    """Exact fp64 scalar recursions for the output-extraction weights."""
    gamma = np.zeros(NEVAL)
    la = np.zeros(NEVAL)
    alpha_y = alpha_z = 1.0
    nu_y = nu_z = 0.0
    for s in range(NSTEP):
        la[2 * s] += -HSTEP
        nu_z += -HSTEP
        gamma *= INVL
        alpha_y *= INVL
        nu_y *= INVL
        gamma += (1.0 - INVL) * la
        alpha_y += (1.0 - INVL) * alpha_z
        nu_y += (1.0 - INVL) * nu_z
        gamma[2 * s + 1] += -INVL * HSTEP
        nu_y += -INVL * HSTEP
    return gamma, alpha_y, nu_y


def _host_tables(W1, b1, u1, W2, b2):
    """All precomputed tensors, fp64 internally."""
    W1 = W1.astype(np.float64)
    W2 = W2.astype(np.float64)
    b1 = b1.astype(np.float64)
    u1 = u1.astype(np.float64)
    b2 = b2.astype(np.float64)

    Mz = -HSTEP * (W1 @ W2)  # [H, H]
    W1b2 = W1 @ b2  # [H]
    l = LCOUP

    def be(s):
        return b1 + (1.0 - s * HSTEP) * u1

    def bp(s):  # beta_odd
        return b1 + (1.0 - (s + 1) * HSTEP) * u1 - (s + 1) * HSTEP * W1b2

    # mzt_pack[p, (k*NBLK+j)*128 + m] = Mz[128*j+m, 128*k+p]
    MzT = Mz.T
    mzt_pack = np.zeros((128, NBLK * NBLK * 128))
    for k in range(NBLK):
        for j in range(NBLK):
            mzt_pack[:, (k * NBLK + j) * 128 : (k * NBLK + j + 1) * 128] = MzT[
                128 * k : 128 * k + 128, 128 * j : 128 * j + 128
            ]

    # rank-2 bias tables: lhsT slice [2, 128] at cols 128*s
    dbz = np.zeros((2, NSTEP * 128))
    dby = np.zeros((2, NSTEP * 128))
    for s in range(NSTEP):
        dz = bp(s) if s == 0 else bp(s) - bp(s - 1)
        for k in range(NBLK):
            dbz[k, s * 128 : (s + 1) * 128] = dz[128 * k : 128 * k + 128]
    for s in range(NSTEP - 1):
        dh = -HSTEP * W1b2 + l * be(s + 1) - (l - 1.0) * bp(s) - be(s)
        if s >= 1:
            # p' reads the Z-bank BEFORE this step's delta; compensate here
            dh = dh + (l - 1.0) * (bp(s) - bp(s - 1))
        for k in range(NBLK):
            dby[k, s * 128 : (s + 1) * 128] = dh[128 * k : 128 * k + 128]
    # col-block NSTEP-1 of dby = init bias l*be(0)
    ib = l * be(0)
    for k in range(NBLK):
        dby[k, (NSTEP - 1) * 128 : NSTEP * 128] = ib[128 * k : 128 * k + 128]

    ind = np.zeros((2, FREE))
    for k in range(NBLK):
        ind[k, k * BSH : (k + 1) * BSH] = 1.0

    dbz0 = dbz[:, 0:128].astype(np.float32)
    dby0 = dby[:, (NSTEP - 1) * 128 : NSTEP * 128].astype(np.float32)

    return dict(
        mzt=mzt_pack.astype(BF16NP),
        mzl=((l - 1.0) * mzt_pack).astype(BF16NP),
        ib16=np.eye(128).astype(BF16NP),
        dbz=dbz.astype(BF16NP),
        dby=dby.astype(BF16NP),
        dbz0=dbz0,
        dby0=dby0,
        ind=ind.astype(np.float32),
        indb=ind.astype(BF16NP),
        w1t=W1.T.astype(np.float32),
        w1tl=(l * W1.T).astype(np.float32),
    )


def _build_kernel():
    """Build the Bass module (same program for every core)."""
    nc = bacc.Bacc("TRN2", target_bir_lowering=False, debug=False)

    y1t_d = [
        nc.dram_tensor(f"y1t{g}", [D, BSH], F32, kind="ExternalInput").ap()
        for g in range(NSH)
    ]
    w1t_d = nc.dram_tensor("w1t", [D, H], F32, kind="ExternalInput").ap()
    w1tl_d = nc.dram_tensor("w1tl", [D, H], F32, kind="ExternalInput").ap()
    mzt_d = nc.dram_tensor("mzt", [128, NBLK * NBLK * 128], BF16, kind="ExternalInput").ap()
    mzl_d = nc.dram_tensor("mzl", [128, NBLK * NBLK * 128], BF16, kind="ExternalInput").ap()
    ib16_d = nc.dram_tensor("ib16", [128, 128], BF16, kind="ExternalInput").ap()
    dbz_d = nc.dram_tensor("dbz", [2, NSTEP * 128], BF16, kind="ExternalInput").ap()
    dby_d = nc.dram_tensor("dby", [2, NSTEP * 128], BF16, kind="ExternalInput").ap()
    dbz0_d = nc.dram_tensor("dbz0", [2, 128], F32, kind="ExternalInput").ap()
    dby0_d = nc.dram_tensor("dby0", [2, 128], F32, kind="ExternalInput").ap()
    ind_d = nc.dram_tensor("ind", [2, FREE], F32, kind="ExternalInput").ap()
    indb_d = nc.dram_tensor("indb", [2, FREE], BF16, kind="ExternalInput").ap()

    ae_out_d = [
        nc.dram_tensor(f"ae_out{g}", [128, ACOLS], BF16, kind="ExternalOutput").ap()
        for g in range(NSH)
    ]
    ao_out_d = [
        nc.dram_tensor(f"ao_out{g}", [128, ACOLS], BF16, kind="ExternalOutput").ap()
        for g in range(NSH)
    ]

    with tile.TileContext(nc) as tc, ExitStack() as ctx:
        consts = ctx.enter_context(tc.tile_pool(name="consts", bufs=1))
        zpool = ctx.enter_context(tc.tile_pool(name="zps", bufs=1, space="PSUM"))
        ypool = ctx.enter_context(tc.tile_pool(name="yps", bufs=2 * NSH, space="PSUM"))
        ppool = ctx.enter_context(tc.tile_pool(name="ptmp", bufs=2))

        # --- prime the tanh activation table early (dep-free) ---
        warm = consts.tile([1, 8], F32, tag="warm")
        nc.vector.memset(warm[:], 0.0)
        nc.scalar.activation(warm[:], warm[:], mybir.ActivationFunctionType.Tanh)

        # --- load constants ---
        def cload(name, shape, dt, dram):
            t = consts.tile(shape, dt, tag=name, name=name)
            nc.sync.dma_start(t[:], dram)
            return t

        y1t = [cload(f"y1t{g}", [D, BSH], F32, y1t_d[g]) for g in range(NSH)]
        w1t = cload("w1t", [D, H], F32, w1t_d)
        w1tl = cload("w1tl", [D, H], F32, w1tl_d)
        mzt = cload("mzt", [128, NBLK * NBLK * 128], BF16, mzt_d)
        mzl = cload("mzl", [128, NBLK * NBLK * 128], BF16, mzl_d)
        ib16 = cload("ib16", [128, 128], BF16, ib16_d)
        dbz = cload("dbz", [2, NSTEP * 128], BF16, dbz_d)
        dby = cload("dby", [2, NSTEP * 128], BF16, dby_d)
        dbz0 = cload("dbz0", [2, 128], F32, dbz0_d)
        dby0 = cload("dby0", [2, 128], F32, dby0_d)
        ind = cload("ind", [2, FREE], F32, ind_d)
        indb = cload("indb", [2, FREE], BF16, indb_d)

        abuf_e = [
            [
                consts.tile([128, CCOLS], BF16, tag=f"abe{g}_{c}", name=f"abe{g}_{c}")
                for c in range(DMA_CHUNKS)
            ]
            for g in range(NSH)
        ]
        abuf_o = [
            [
                consts.tile([128, CCOLS], BF16, tag=f"abo{g}_{c}", name=f"abo{g}_{c}")
                for c in range(DMA_CHUNKS)
            ]
            for g in range(NSH)
        ]

        def mzt_blk(k, j):
            base = (k * NBLK + j) * 128
            return mzt[:, base : base + 128]

        def mzl_blk(k, j):
            base = (k * NBLK + j) * 128
            return mzl[:, base : base + 128]

        # --- per-chain state ---
        st = []
        for g in range(NSH):
            z_ps = zpool.tile([128, FREE], F32, tag=f"z{g}", name=f"z{g}")
            # init Z-bank = W1 @ y1 + beta_odd(0)
            for j in range(NBLK):
                nc.tensor.matmul(
                    z_ps[:, j * BSH : (j + 1) * BSH],
                    w1t[:, 128 * j : 128 * j + 128],
                    y1t[g][:],
                    start=(j == 0),
                    stop=False,
                )
            nc.tensor.matmul(z_ps[:], dbz0[:], ind[:], start=False, stop=True)

            # init Y-bank = l*(W1 @ y1) + l*be(0)
            y_cur = ypool.tile([128, FREE], F32, tag="y", name=f"y{g}init")
            for j in range(NBLK):
                nc.tensor.matmul(
                    y_cur[:, j * BSH : (j + 1) * BSH],
                    w1tl[:, 128 * j : 128 * j + 128],
                    y1t[g][:],
                    start=(j == 0),
                    stop=False,
                )
            nc.tensor.matmul(y_cur[:], dby0[:], ind[:], start=False, stop=True)
            st.append({"z": z_ps, "y": y_cur})

        for s in range(NSTEP):
            last = s == NSTEP - 1
            chunk, cstep = divmod(s, CSTEPS)
            ecol = cstep * FREE

            for g in range(NSH):
                z_ps = st[g]["z"]
                y_cur = st[g]["y"]

                if not last:
                    # t1 = (l-1) * Zbank_pre (before this step's delta-MM)
                    t_t = ppool.tile([128, FREE], F32, tag=f"t{g}", name=f"t{g}_{s}")
                    nc.vector.tensor_scalar_mul(t_t[:], z_ps[:], LCOUP - 1.0)

                if s > 0:
                    nc.tensor.matmul(
                        z_ps[:], dbz[:, s * 128 : (s + 1) * 128], indb[:],
                        start=False, stop=False, skip_group_check=True,
                    )

                # --- even eval ---
                a_even = abuf_e[g][chunk][:, ecol : ecol + FREE]
                nc.scalar.activation(
                    a_even[:], y_cur[:], mybir.ActivationFunctionType.Tanh,
                    scale=INVL,
                )

                if not last:
                    # p = inv_l Ybank + t1, compensated split p = hi + lo
                    p_t = ppool.tile([128, FREE], F32, tag=f"p{g}", name=f"p{g}_{s}")
                    nc.vector.scalar_tensor_tensor(
                        p_t[:], y_cur[:], INVL, t_t[:],
                        mybir.AluOpType.mult, mybir.AluOpType.add,
                    )
                    p_hi = ppool.tile([128, FREE], BF16, tag=f"phi{g}", name=f"phi{g}_{s}")
                    nc.vector.tensor_copy(p_hi[:], p_t[:])
                    p_lo = ppool.tile([128, FREE], BF16, tag=f"plo{g}", name=f"plo{g}_{s}")
                    nc.vector.scalar_tensor_tensor(
                        p_lo[:], p_hi[:], -1.0, p_t[:],
                        mybir.AluOpType.mult, mybir.AluOpType.add,
                    )

                # --- Z += Mz @ a_even ---
                for j in range(NBLK):
                    for k in range(NBLK):
                        nc.tensor.matmul(
                            z_ps[:, j * BSH : (j + 1) * BSH],
                            mzt_blk(k, j),
                            a_even[:, k * BSH : (k + 1) * BSH],
                            start=False,
                            stop=False,
                            skip_group_check=True,
                        )

                if not last:
                    # next Y-bank: bias, then the a_even-driven part and the
                    # carry (all independent of a_odd -> run during odd ACT)
                    y_next = ypool.tile([128, FREE], F32, tag="y", name=f"y{g}_{s}")
                    nc.tensor.matmul(
                        y_next[:], dby[:, s * 128 : (s + 1) * 128], indb[:],
                        start=True, stop=False,
                    )
                    for j in range(NBLK):
                        for k in range(NBLK):
                            nc.tensor.matmul(
                                y_next[:, j * BSH : (j + 1) * BSH],
                                mzl_blk(k, j),
                                a_even[:, k * BSH : (k + 1) * BSH],
                                start=False,
                                stop=False,
                            )
                    nc.tensor.matmul(y_next[:], ib16[:], p_hi[:], start=False, stop=False)
                    nc.tensor.matmul(y_next[:], ib16[:], p_lo[:], start=False, stop=False)
                    st[g]["y_next"] = y_next

                # --- odd eval ---
                a_odd = abuf_o[g][chunk][:, ecol : ecol + FREE]
                nc.scalar.activation(
                    a_odd[:], z_ps[:], mybir.ActivationFunctionType.Tanh, scale=1.0
                )

                if not last:
                    y_next = st[g]["y_next"]
                    for j in range(NBLK):
                        for k in range(NBLK):
                            nc.tensor.matmul(
                                y_next[:, j * BSH : (j + 1) * BSH],
                                mzt_blk(k, j),
                                a_odd[:, k * BSH : (k + 1) * BSH],
                                start=False,
                                stop=(j == NBLK - 1 and k == NBLK - 1),
                            )
                    st[g]["y"] = y_next

            if (s + 1) % CSTEPS == 0:
                c0 = chunk * CCOLS
                for g in range(NSH):
                    nc.sync.dma_start(
                        ae_out_d[g][:, c0 : c0 + CCOLS], abuf_e[g][chunk][:]
                    )
                    nc.sync.dma_start(
                        ao_out_d[g][:, c0 : c0 + CCOLS], abuf_o[g][chunk][:]
                    )

    nc.compile()
    return nc


_CACHE = {}


def _get_kernel():
    if "nc" not in _CACHE:
        _CACHE["nc"] = _build_kernel()
    return _CACHE["nc"]


def kernel(y1, W1, b1, u1, W2, b2, _trace=False, _trace_kwargs=None):
    y1 = np.asarray(y1)
    in_dtype = y1.dtype
    W1_ = np.asarray(W1, dtype=np.float64)
    W2_ = np.asarray(W2, dtype=np.float64)
    b2_ = np.asarray(b2, dtype=np.float64)
    tabs = _host_tables(
        np.asarray(W1), np.asarray(b1), np.asarray(u1), np.asarray(W2), np.asarray(b2)
    )

    nc = _get_kernel()

    shared = {k: tabs[k] for k in SHARED_INPUTS}
    in_maps = []
    for c in range(NCORES):
        m = dict(shared)
        for g in range(NSH):
            r0 = c * BS + g * BSH
            shard = y1[r0 : r0 + BSH].astype(np.float32)  # [BSH, D]
            m[f"y1t{g}"] = np.ascontiguousarray(shard.T)  # [D, BSH]
        in_maps.append(m)

    kw = {}
    if _trace:
        kw["trace"] = True
        if _trace_kwargs:
            kw.update(_trace_kwargs)
    res = run_bass_kernel_spmd(nc, in_maps, core_ids=list(range(NCORES)), **kw)

    # --- exact host-side output extraction ---
    gamma, c_y, c_b = _coefficients()
    cvec = np.sum(W1_ * W2_.T, axis=1)  # diag(W1@W2)
    sum_c = float(np.sum(cvec))

    out = np.zeros((B, D + 1), dtype=np.float32)
    for c in range(NCORES):
        for g in range(NSH):
            ae = np.asarray(res.results[c][f"ae_out{g}"]).astype(np.float64)
            ao = np.asarray(res.results[c][f"ao_out{g}"]).astype(np.float64)
            ae = ae.reshape(128, NSTEP, NBLK, BSH)  # [p, s, blk, b]
            ao = ao.reshape(128, NSTEP, NBLK, BSH)
            ae = np.moveaxis(ae, (2, 0), (1, 2)).reshape(NSTEP, H, BSH)  # [s,h,b]
            ao = np.moveaxis(ao, (2, 0), (1, 2)).reshape(NSTEP, H, BSH)

            S = np.einsum("s,shb->hb", gamma[0::2], ae) + np.einsum(
                "s,shb->hb", gamma[1::2], ao
            )
            r0 = c * BS + g * BSH
            shard = y1[r0 : r0 + BSH].astype(np.float64)  # [BSH, D]
            y_fin = c_y * shard + (W2_ @ S).T + c_b * b2_[None, :]
            ptr = np.einsum("h,shb->b", cvec, ae**2)
            i_fin = HSTEP * (NSTEP * sum_c - ptr)
            out[r0 : r0 + BSH, :D] = y_fin.astype(np.float32)
            out[r0 : r0 + BSH, D] = i_fin.astype(np.float32)

    if _trace:
        return out.astype(in_dtype, copy=False), res
    return out.astype(in_dtype, copy=False)

